# revision 1
# baseline (speedup 1.0000x reference)
import numpy as np

# nn_LocalDynamics GNN message passing.
# delta[n] = sum_e tanh(fMLP(inp_e))[addr_from=n] + tanh(tMLP(inp_e))[addr_to=n]
# out = tanh(delta).  inp_e = [h[from], h[to], x_e, hg, xg, t] (153 dims).
N = 100_000
E = 800_000
D = 64
H = 128
NCORES = 8
PAIR = 1024                      # edges per device iteration
EPC = E // NCORES                # 100000 edges per core
EPAD = ((EPC + PAIR - 1) // PAIR) * PAIR   # 100352


def _scatter_add(delta, idx, vals):
    o = np.argsort(idx, kind="stable")
    si = idx[o]
    sv = vals[o]
    starts = np.flatnonzero(np.r_[True, si[1:] != si[:-1]])
    sums = np.add.reduceat(sv, starts, axis=0)
    np.add.at(delta, si[starts], sums)


def _host_post(af, at, d_from, d_to, n_nodes):
    delta = np.zeros((n_nodes, D), dtype=np.float32)
    _scatter_add(delta, af, d_from)
    _scatter_add(delta, at, d_to)
    return np.tanh(delta).astype(np.float32)


def _mlp_np(x, W0, b0, W1, b1, W2, b2):
    h = np.tanh(x @ W0 + b0)
    h = np.tanh(h @ W1 + b1)
    return h @ W2 + b2


def _kernel_numpy(addr_from, addr_to, h_local, h_global, x_local, x_global, t,
                  f_W0, f_b0, f_W1, f_b1, f_W2, f_b2,
                  t_W0, t_b0, t_W1, t_b1, t_W2, t_b2):
    af = np.asarray(addr_from).astype(np.int64)
    at = np.asarray(addr_to).astype(np.int64)
    h_local = np.asarray(h_local, dtype=np.float32)
    x_local = np.asarray(x_local, dtype=np.float32)
    const = np.concatenate([np.asarray(h_global, np.float32).ravel(),
                            np.asarray(x_global, np.float32).ravel(),
                            np.asarray(t, np.float32).ravel()])
    ne = af.shape[0]
    d_from = np.empty((ne, D), np.float32)
    d_to = np.empty((ne, D), np.float32)
    CH = 100_000
    for s in range(0, ne, CH):
        e = min(s + CH, ne)
        inp = np.concatenate([h_local[af[s:e]], h_local[at[s:e]], x_local[s:e],
                              np.broadcast_to(const, (e - s, 21))], axis=1).astype(np.float32)
        d_from[s:e] = np.tanh(_mlp_np(inp, f_W0, f_b0, f_W1, f_b1, f_W2, f_b2))
        d_to[s:e] = np.tanh(_mlp_np(inp, t_W0, t_b0, t_W1, t_b1, t_W2, t_b2))
    return _host_post(af, at, d_from, d_to, h_local.shape[0])


_BASS_CACHE = {}


def _build_bass():
    import concourse.bass as bass
    import concourse.mybir as mybir
    import concourse.tile as tile

    # walrus in this env rejects Drain instructions carrying >1 sem wait;
    # move each wait onto its own sync nop before the drain.
    def _patched(self, tick_clock, wait_clock):
        from concourse.tile import ScopedClock
        nop0 = self.nc.sync.nop(nofuse=True)
        wait_clock.add_sem_waits(nop0.ins, ScopedClock({None: tick_clock.global_clock}))
        si = nop0.ins.sync_info
        if si is not None and si.on_wait and len(si.on_wait) > 1:
            waits = list(si.on_wait)
            si.on_wait = waits[:1]
            for w in waits[1:]:
                n = self.nc.sync.nop(nofuse=True)
                n.ins.sync_info = mybir.SyncInfo(on_wait=[w], on_update=[])
        self.nc.sync.drain()
        self.nc.all_engine_barrier()
        popped = self.nc._tile_sem_poison_stack.pop()
        assert popped is self._sem_poison
        self.nc.clear_and_free_semaphores(list(self.sems.allocated().values()))
        self.nc.all_engine_barrier()

    tile.TileContext._drain_and_barrier = _patched

    f32 = mybir.dt.float32
    f32r = mybir.dt.float32r
    nc = bass.Bass()
    inpa = nc.dram_tensor("inpa", [128, EPAD], f32, kind="ExternalInput")
    inpb = nc.dram_tensor("inpb", [4, EPAD], f32, kind="ExternalInput")
    wts = {}
    for p in ("f", "t"):
        wts[p + "w0a"] = nc.dram_tensor(p + "w0a", [128, H], f32, kind="ExternalInput")
        wts[p + "w0b"] = nc.dram_tensor(p + "w0b", [4, H], f32, kind="ExternalInput")
        wts[p + "w1"] = nc.dram_tensor(p + "w1", [H, H], f32, kind="ExternalInput")
        wts[p + "w2"] = nc.dram_tensor(p + "w2", [H, D], f32, kind="ExternalInput")
        wts[p + "b0"] = nc.dram_tensor(p + "b0", [H, 1], f32, kind="ExternalInput")
        wts[p + "b1"] = nc.dram_tensor(p + "b1", [H, 1], f32, kind="ExternalInput")
        wts[p + "b2"] = nc.dram_tensor(p + "b2", [D, 1], f32, kind="ExternalInput")
    douts = {p: nc.dram_tensor("d" + p, [D, EPAD], f32, kind="ExternalOutput")
             for p in ("f", "t")}

    Tanh = mybir.ActivationFunctionType.Tanh
    with tile.TileContext(nc) as tc:
        with tc.tile_pool(name="wpool", bufs=1) as wp, \
             tc.tile_pool(name="io", bufs=3) as io, \
             tc.tile_pool(name="act", bufs=2) as ap_, \
             tc.tile_pool(name="ps01", bufs=1, space="PSUM") as ps01, \
             tc.tile_pool(name="ps2", bufs=2, space="PSUM") as ps2:
            wt = {}
            for k, dr in wts.items():
                sh = list(dr.shape)
                tl = wp.tile(sh, f32, tag="w" + k)
                nc.sync.dma_start(out=tl[:], in_=dr[:])
                wt[k] = tl
            for it in range(EPAD // PAIR):
                sl = slice(it * PAIR, (it + 1) * PAIR)
                ra = io.tile([128, PAIR], f32, tag="ra")
                rb = io.tile([4, PAIR], f32, tag="rb")
                nc.sync.dma_start(out=ra[:], in_=inpa[:, sl])
                nc.sync.dma_start(out=rb[:], in_=inpb[:, sl])
                for p in ("f", "t"):
                    ps0 = ps01.tile([128, PAIR], f32, tag="ps0")
                    for hh in range(2):
                        hs = slice(hh * 512, (hh + 1) * 512)
                        nc.tensor.matmul(out=ps0[:, hs],
                                         lhsT=wt[p + "w0a"][:],
                                         rhs=ra[:, hs],
                                         start=True, stop=False)
                        nc.tensor.matmul(out=ps0[:, hs],
                                         lhsT=wt[p + "w0b"][:],
                                         rhs=rb[:, hs],
                                         start=False, stop=True)
                    h1 = ap_.tile([128, PAIR], f32, tag="h1")
                    nc.scalar.activation(h1[:], ps0[:], Tanh, bias=wt[p + "b0"][:, 0:1])
                    ps1 = ps01.tile([128, PAIR], f32, tag="ps1")
                    for hh in range(2):
                        hs = slice(hh * 512, (hh + 1) * 512)
                        nc.tensor.matmul(out=ps1[:, hs],
                                         lhsT=wt[p + "w1"][:],
                                         rhs=h1[:, hs],
                                         start=True, stop=True)
                    h2 = ap_.tile([128, PAIR], f32, tag="h2")
                    nc.scalar.activation(h2[:], ps1[:], Tanh, bias=wt[p + "b1"][:, 0:1])
                    psd = ps2.tile([D, PAIR], f32, tag="psd")
                    for hh in range(2):
                        hs = slice(hh * 512, (hh + 1) * 512)
                        nc.tensor.matmul(out=psd[:, hs],
                                         lhsT=wt[p + "w2"][:],
                                         rhs=h2[:, hs],
                                         start=True, stop=True)
                    dt_ = ap_.tile([D, PAIR], f32, tag="dt")
                    nc.scalar.activation(dt_[:], psd[:], Tanh, bias=wt[p + "b2"][:, 0:1])
                    nc.sync.dma_start(out=douts[p][:, sl], in_=dt_[:])

    # this walrus rejects any compute instruction carrying >1 sem wait;
    # hoist extra waits onto same-engine nops placed just before it.
    ctr = 0
    for bb in nc.main_func.blocks:
        new = []
        for ins in bb.instructions:
            si = getattr(ins, "sync_info", None)
            if si is not None and si.on_wait and len(si.on_wait) > 1:
                waits = list(si.on_wait)
                si.on_wait = [waits[-1]]
                for w in waits[:-1]:
                    ctr += 1
                    nop = mybir.InstNoOp(
                        name=f"wsplit-{ctr}", engine=ins.engine, ins=[], outs=[],
                        sync_info=mybir.SyncInfo(on_wait=[w], on_update=[]))
                    new.append(nop)
            new.append(ins)
        bb.instructions[:] = new
    return nc


def _kernel_bass(addr_from, addr_to, h_local, h_global, x_local, x_global, t,
                 f_W0, f_b0, f_W1, f_b1, f_W2, f_b2,
                 t_W0, t_b0, t_W1, t_b1, t_W2, t_b2, trace=False):
    import sys
    if "/opt/trn_rl_repo" not in sys.path:
        sys.path.insert(0, "/opt/trn_rl_repo")
    from concourse.bass_utils import run_bass_kernel_spmd

    af = np.asarray(addr_from).astype(np.int64)
    at = np.asarray(addr_to).astype(np.int64)
    h_local = np.ascontiguousarray(np.asarray(h_local, np.float32))
    x_local = np.asarray(x_local, np.float32)
    const = np.concatenate([np.asarray(h_global, np.float32).ravel(),
                            np.asarray(x_global, np.float32).ravel(),
                            np.asarray(t, np.float32).ravel()])  # [21]

    if "nc" not in _BASS_CACHE:
        _BASS_CACHE["nc"] = _build_bass()
    nc = _BASS_CACHE["nc"]

    weights = {}
    for p, W0, b0, W1, b1, W2, b2 in (
        ("f", f_W0, f_b0, f_W1, f_b1, f_W2, f_b2),
        ("t", t_W0, t_b0, t_W1, t_b1, t_W2, t_b2),
    ):
        W0 = np.asarray(W0, np.float32)
        b0eff = np.asarray(b0, np.float32) + const @ W0[132:153]
        weights[p + "w0a"] = np.ascontiguousarray(W0[0:128])
        weights[p + "w0b"] = np.ascontiguousarray(W0[128:132])
        weights[p + "w1"] = np.asarray(W1, np.float32)
        weights[p + "w2"] = np.asarray(W2, np.float32)
        weights[p + "b0"] = b0eff.reshape(H, 1)
        weights[p + "b1"] = np.asarray(b1, np.float32).reshape(H, 1)
        weights[p + "b2"] = np.asarray(b2, np.float32).reshape(D, 1)

    in_maps = []
    for c in range(NCORES):
        s, e = c * EPC, (c + 1) * EPC
        ia = np.zeros((128, EPAD), np.float32)
        ia[0:64, :EPC] = h_local[af[s:e]].T
        ia[64:128, :EPC] = h_local[at[s:e]].T
        ib = np.zeros((4, EPAD), np.float32)
        ib[:, :EPC] = x_local[s:e].T
        m = {"inpa": ia, "inpb": ib}
        m.update(weights)
        in_maps.append(m)

    res = run_bass_kernel_spmd(nc, in_maps, core_ids=list(range(NCORES)),
                               trace=trace)
    d_from = np.concatenate(
        [res.results[c]["df"][:, :EPC].T for c in range(NCORES)], axis=0)
    d_to = np.concatenate(
        [res.results[c]["dt"][:, :EPC].T for c in range(NCORES)], axis=0)
    out = _host_post(af, at, d_from, d_to, h_local.shape[0])
    if trace:
        return out, res
    return out


def kernel(**inputs):
    try:
        return _kernel_bass(**inputs)
    except Exception:
        import traceback
        traceback.print_exc()
        return _kernel_numpy(**inputs)



# revision 8
# speedup vs baseline: 11.8577x; 11.8577x over previous
import numpy as np

# nn_LocalDynamics GNN message passing, full on-device pipeline.
#   delta[n] = sum_e tanh(fMLP(inp_e))[addr_from=n] + tanh(tMLP(inp_e))[addr_to=n]
#   out = tanh(delta);  inp_e = [h[from], h[to], x_e, hg, xg, t] (153 dims)
# Strategy: sort edges by destination node on host, shard by node range
# (12544 nodes/core).  Each core AllGathers h, gathers h-rows per edge via
# indirect DMA, runs the MLPs, and aggregates into its own output slice with
# eq-matrix matmuls (segmented sum) -- no scatter, no reduce collective.
N = 100_000
NPAD = 100_352
NPC = 12_544          # nodes per core
E = 800_000
D = 64
H = 128
WPC = 98              # windows (128 nodes) per core
CAP = 10              # chunks of 128 elems per (window, stream)
CH = 128
SW = CAP * CH         # 1280 elems per (window, stream)
TOTW = 2 * SW
TOT = WPC * TOTW      # 250880 padded stream elems per core
NCORES = 8


def _scatter_add(delta, idx, vals):
    o = np.argsort(idx, kind="stable")
    si = idx[o]
    sv = vals[o]
    starts = np.flatnonzero(np.r_[True, si[1:] != si[:-1]])
    sums = np.add.reduceat(sv, starts, axis=0)
    np.add.at(delta, si[starts], sums)


def _mlp_np(x, W0, b0, W1, b1, W2, b2):
    h = np.tanh(x @ W0 + b0)
    h = np.tanh(h @ W1 + b1)
    return h @ W2 + b2


def _kernel_numpy(addr_from, addr_to, h_local, h_global, x_local, x_global, t,
                  f_W0, f_b0, f_W1, f_b1, f_W2, f_b2,
                  t_W0, t_b0, t_W1, t_b1, t_W2, t_b2):
    af = np.asarray(addr_from).astype(np.int64)
    at = np.asarray(addr_to).astype(np.int64)
    h_local = np.asarray(h_local, dtype=np.float32)
    x_local = np.asarray(x_local, dtype=np.float32)
    const = np.concatenate([np.asarray(h_global, np.float32).ravel(),
                            np.asarray(x_global, np.float32).ravel(),
                            np.asarray(t, np.float32).ravel()])
    ne = af.shape[0]
    d_from = np.empty((ne, D), np.float32)
    d_to = np.empty((ne, D), np.float32)
    CHK = 100_000
    for s in range(0, ne, CHK):
        e = min(s + CHK, ne)
        inp = np.concatenate([h_local[af[s:e]], h_local[at[s:e]], x_local[s:e],
                              np.broadcast_to(const, (e - s, 21))], axis=1
                             ).astype(np.float32)
        d_from[s:e] = np.tanh(_mlp_np(inp, f_W0, f_b0, f_W1, f_b1, f_W2, f_b2))
        d_to[s:e] = np.tanh(_mlp_np(inp, t_W0, t_b0, t_W1, t_b1, t_W2, t_b2))
    delta = np.zeros((h_local.shape[0], D), dtype=np.float32)
    _scatter_add(delta, af, d_from)
    _scatter_add(delta, at, d_to)
    return np.tanh(delta).astype(np.float32)


def _prep(af, at, x):
    """Sort both streams by destination, pad into the fixed window layout.
    Returns per-core device arrays:
      af3/at3 [8, 98, 128, 20] int32, dr3 [8, 98, 128, 20] f32,
      x3 [8, 98, 4, 2560] f32.
    Chunk columns 0:10 = f-stream, 10:20 = t-stream; pad elems dr=-1, af/at=0.
    """
    AF = np.zeros((NCORES, TOT), np.int32)
    AT = np.zeros((NCORES, TOT), np.int32)
    DR = np.full((NCORES, TOT), -1.0, np.float32)
    X = np.zeros((NCORES, TOT, 4), np.float32)
    for si, dest in ((0, af), (1, at)):
        o = np.argsort(dest, kind="stable")
        ds_ = dest[o]
        w = ds_ >> 7
        cnts = np.bincount(w, minlength=NPAD // 128)
        if cnts.max() > SW:
            raise OverflowError(f"window count {cnts.max()} > {SW}")
        starts = np.concatenate([[0], np.cumsum(cnts)[:-1]])
        cc = np.arange(E, dtype=np.int64) - starts[w]
        core = w // WPC
        wl = w - core * WPC
        off = wl * TOTW + si * SW + cc
        AF[core, off] = af[o]
        AT[core, off] = at[o]
        DR[core, off] = (ds_ & 127).astype(np.float32)
        X[core, off] = x[o]
    af2 = np.ascontiguousarray(
        AF.reshape(NCORES, TOT // CH, CH).transpose(0, 2, 1))
    at2 = np.ascontiguousarray(
        AT.reshape(NCORES, TOT // CH, CH).transpose(0, 2, 1))
    dr3 = np.ascontiguousarray(
        DR.reshape(NCORES, WPC, 2 * CAP, CH).transpose(0, 1, 3, 2))
    x3 = np.ascontiguousarray(
        X.reshape(NCORES, WPC, TOTW, 4).transpose(0, 1, 3, 2))
    return af2, at2, dr3, x3


_BASS_CACHE = {}


def _patch_walrus(bass, mybir, tile):
    # walrus in this env rejects Drain instructions carrying >1 sem wait;
    # move each wait onto its own sync nop before the drain.
    def _patched(self, tick_clock, wait_clock):
        from concourse.tile import ScopedClock
        nop0 = self.nc.sync.nop(nofuse=True)
        wait_clock.add_sem_waits(nop0.ins, ScopedClock({None: tick_clock.global_clock}))
        si = nop0.ins.sync_info
        if si is not None and si.on_wait and len(si.on_wait) > 1:
            waits = list(si.on_wait)
            si.on_wait = waits[:1]
            for w in waits[1:]:
                n = self.nc.sync.nop(nofuse=True)
                n.ins.sync_info = mybir.SyncInfo(on_wait=[w], on_update=[])
        self.nc.sync.drain()
        self.nc.all_engine_barrier()
        popped = self.nc._tile_sem_poison_stack.pop()
        assert popped is self._sem_poison
        self.nc.clear_and_free_semaphores(list(self.sems.allocated().values()))
        self.nc.all_engine_barrier()

    tile.TileContext._drain_and_barrier = _patched


def _split_multi_waits(nc, mybir):
    # this walrus rejects any compute instruction carrying >1 sem wait;
    # hoist extra waits onto same-engine nops placed just before it.
    ctr = 0
    for bb in nc.main_func.blocks:
        new = []
        for ins in bb.instructions:
            si = getattr(ins, "sync_info", None)
            if si is not None and si.on_wait and len(si.on_wait) > 1:
                waits = list(si.on_wait)
                si.on_wait = [waits[-1]]
                for w in waits[:-1]:
                    ctr += 1
                    nop = mybir.InstNoOp(
                        name=f"wsplit-{ctr}", engine=ins.engine, ins=[], outs=[],
                        sync_info=mybir.SyncInfo(on_wait=[w], on_update=[]))
                    new.append(nop)
            new.append(ins)
        bb.instructions[:] = new


def _build_bass():
    import concourse.bass as bass
    import concourse.mybir as mybir
    import concourse.tile as tile
    from concourse.bass import ds, IndirectOffsetOnAxis

    _patch_walrus(bass, mybir, tile)

    f32 = mybir.dt.float32
    i32 = mybir.dt.int32
    nc = bass.Bass(num_devices=NCORES)

    h_in = nc.dram_tensor("hsh", [NPC, D], f32, kind="ExternalInput")
    af2 = nc.dram_tensor("af2", [CH, WPC * 2 * CAP], i32, kind="ExternalInput")
    at2 = nc.dram_tensor("at2", [CH, WPC * 2 * CAP], i32, kind="ExternalInput")
    dr3 = nc.dram_tensor("dr3", [WPC, CH, 2 * CAP], f32, kind="ExternalInput")
    x3 = nc.dram_tensor("x3", [WPC, 4, TOTW], f32, kind="ExternalInput")
    wts = {}
    for p in ("f", "t"):
        wts[p + "w0a"] = nc.dram_tensor(p + "w0a", [128, H], f32, kind="ExternalInput")
        wts[p + "w0b"] = nc.dram_tensor(p + "w0b", [4, H], f32, kind="ExternalInput")
        wts[p + "w1"] = nc.dram_tensor(p + "w1", [H, H], f32, kind="ExternalInput")
        wts[p + "w2"] = nc.dram_tensor(p + "w2", [H, D], f32, kind="ExternalInput")
        wts[p + "b0"] = nc.dram_tensor(p + "b0", [H, 1], f32, kind="ExternalInput")
        wts[p + "b1"] = nc.dram_tensor(p + "b1", [H, 1], f32, kind="ExternalInput")
        wts[p + "b2"] = nc.dram_tensor(p + "b2", [D, 1], f32, kind="ExternalInput")
    outp = nc.dram_tensor("out3", [WPC, CH, D], f32, kind="ExternalOutput")

    h_int = nc.dram_tensor("h_int", [NPC, D], f32, kind="Internal")
    h_full = nc.dram_tensor("h_full", [NPAD, D], f32, kind="Internal",
                            addr_space="Shared")
    # DRAM staging for gathered h-rows: SWDGE indirect DMAs must stay outside
    # For_i (this walrus can't encode the loop's SWDGE sem-reset), so an
    # unrolled gather phase writes here and the loop reads it back with HWDGE.
    gstage = nc.dram_tensor("gstage", [WPC, CH, 2 * CAP, CH], f32,
                            kind="Internal")
    ident_dr = nc.inline_tensor(np.eye(CH, dtype=np.float32), name="ident")
    iota_dr = nc.inline_tensor(
        np.ascontiguousarray(np.broadcast_to(
            np.arange(CH, dtype=np.float32), (CH, CH))), name="iotam")

    Tanh = mybir.ActivationFunctionType.Tanh
    with tile.TileContext(nc) as tc:
        with tc.tile_pool(name="wp", bufs=1) as wp, \
             tc.tile_pool(name="io", bufs=2) as io, \
             tc.tile_pool(name="act", bufs=2) as ap_, \
             tc.tile_pool(name="psm", bufs=2, space="PSUM") as psm, \
             tc.tile_pool(name="psw", bufs=1, space="PSUM") as psw:
            wt = {}
            for k, drh in wts.items():
                tl = wp.tile(list(drh.shape), f32, tag="w" + k)
                nc.sync.dma_start(out=tl[:], in_=drh[:])
                wt[k] = tl
            ident = wp.tile([CH, CH], f32, tag="ident")
            nc.sync.dma_start(out=ident[:], in_=ident_dr[:])
            iota_sb = wp.tile([CH, CH], f32, tag="iota")
            nc.sync.dma_start(out=iota_sb[:], in_=iota_dr[:])

            nc.sync.dma_start(out=h_int[:], in_=h_in[:])
            nc.gpsimd.collective_compute(
                "AllGather", mybir.AluOpType.bypass,
                replica_groups=[list(range(NCORES))],
                ins=[h_int[:]], outs=[h_full[:]])

            af_all = wp.tile([CH, WPC * 2 * CAP], i32, tag="afall")
            at_all = wp.tile([CH, WPC * 2 * CAP], i32, tag="atall")
            nc.sync.dma_start(out=af_all[:], in_=af2[:])
            nc.sync.dma_start(out=at_all[:], in_=at2[:])

            for w in range(WPC):
                g2 = io.tile([CH, 2 * CAP, CH], f32, tag="g2")
                for c in range(2 * CAP):
                    col = w * 2 * CAP + c
                    # the vector-indirect DMA only supports one index per
                    # partition per instruction (idx [128,1] -> out [128,64])
                    nc.gpsimd.indirect_dma_start(
                        out=g2[:, c, 0:64], out_offset=None, in_=h_full[:],
                        in_offset=IndirectOffsetOnAxis(
                            ap=af_all[:, col:col + 1], axis=0))
                    nc.gpsimd.indirect_dma_start(
                        out=g2[:, c, 64:128], out_offset=None, in_=h_full[:],
                        in_offset=IndirectOffsetOnAxis(
                            ap=at_all[:, col:col + 1], axis=0))
                nc.sync.dma_start(out=gstage[w], in_=g2[:])

            with tc.For_i(0, WPC, 1) as iv:
                dr_sb = io.tile([CH, 2 * CAP], f32, tag="dr")
                x_sb = io.tile([4, TOTW], f32, tag="x")
                gld = io.tile([CH, 2 * CAP, CH], f32, tag="gld")
                nc.sync.dma_start(out=dr_sb[:], in_=dr3[ds(iv, 1)].squeeze(0))
                nc.sync.dma_start(out=x_sb[:], in_=x3[ds(iv, 1)].squeeze(0))
                nc.sync.dma_start(out=gld[:], in_=gstage[ds(iv, 1)].squeeze(0))
                win_ps = psw.tile([CH, D], f32, tag="win")
                for si, p in enumerate(("f", "t")):
                    inpT = psm.tile([CH, SW], f32, tag="mlp")
                    for k in range(CAP):
                        nc.tensor.transpose(
                            out=inpT[:, k * CH:(k + 1) * CH],
                            in_=gld[:, si * CAP + k, :], identity=ident[:])
                    inp_sb = ap_.tile([CH, SW], f32, tag="inp")
                    nc.vector.tensor_copy(out=inp_sb[:], in_=inpT[:])
                    ps0 = psm.tile([CH, SW], f32, tag="mlp")
                    for lo in range(0, SW, 512):
                        hi = min(lo + 512, SW)
                        nc.tensor.matmul(out=ps0[:, lo:hi], lhsT=wt[p + "w0a"][:],
                                         rhs=inp_sb[:, lo:hi], start=True, stop=False)
                        nc.tensor.matmul(out=ps0[:, lo:hi], lhsT=wt[p + "w0b"][:],
                                         rhs=x_sb[:, si * SW + lo:si * SW + hi],
                                         start=False, stop=True)
                    h1 = ap_.tile([CH, SW], f32, tag="h1")
                    nc.scalar.activation(h1[:], ps0[:], Tanh, bias=wt[p + "b0"][:, 0:1])
                    ps1 = psm.tile([CH, SW], f32, tag="mlp")
                    for lo in range(0, SW, 512):
                        hi = min(lo + 512, SW)
                        nc.tensor.matmul(out=ps1[:, lo:hi], lhsT=wt[p + "w1"][:],
                                         rhs=h1[:, lo:hi], start=True, stop=True)
                    h2 = ap_.tile([CH, SW], f32, tag="h2")
                    nc.scalar.activation(h2[:], ps1[:], Tanh, bias=wt[p + "b1"][:, 0:1])
                    psd = psm.tile([CH, SW], f32, tag="mlp")
                    for lo in range(0, SW, 512):
                        hi = min(lo + 512, SW)
                        nc.tensor.matmul(out=psd[0:D, lo:hi], lhsT=wt[p + "w2"][:],
                                         rhs=h2[:, lo:hi], start=True, stop=True)
                    d_fm = ap_.tile([D, SW], f32, tag="dfm")
                    nc.scalar.activation(d_fm[:], psd[0:D, :], Tanh,
                                         bias=wt[p + "b2"][:, 0:1])
                    dT = psm.tile([CH, SW], f32, tag="mlp")
                    for k in range(CAP):
                        nc.tensor.transpose(
                            out=dT[:, k * D:(k + 1) * D],
                            in_=d_fm[:, k * CH:(k + 1) * CH],
                            identity=ident[0:64, 0:64])
                    d_em = ap_.tile([CH, CAP * D], f32, tag="dem")
                    nc.vector.tensor_copy(out=d_em[:], in_=dT[:, 0:CAP * D])
                    eq_sb = ap_.tile([CH, SW], f32, tag="eq")
                    for k in range(CAP):
                        col = si * CAP + k
                        nc.vector.tensor_tensor(
                            out=eq_sb[:, k * CH:(k + 1) * CH],
                            in0=dr_sb[:, col:col + 1].to_broadcast([CH, CH]),
                            in1=iota_sb[:], op=mybir.AluOpType.is_equal)
                    for k in range(CAP):
                        nc.tensor.matmul(
                            out=win_ps[:], lhsT=eq_sb[:, k * CH:(k + 1) * CH],
                            rhs=d_em[:, k * D:(k + 1) * D],
                            start=(si == 0 and k == 0),
                            stop=(si == 1 and k == CAP - 1))
                out_sb = ap_.tile([CH, D], f32, tag="out")
                nc.scalar.activation(out_sb[:], win_ps[:], Tanh)
                nc.sync.dma_start(out=outp[ds(iv, 1)].squeeze(0), in_=out_sb[:])

    _split_multi_waits(nc, mybir)
    return nc


def _kernel_bass(addr_from, addr_to, h_local, h_global, x_local, x_global, t,
                 f_W0, f_b0, f_W1, f_b1, f_W2, f_b2,
                 t_W0, t_b0, t_W1, t_b1, t_W2, t_b2, trace=False):
    import sys
    if "/opt/trn_rl_repo" not in sys.path:
        sys.path.insert(0, "/opt/trn_rl_repo")
    from concourse.bass_utils import run_bass_kernel_spmd

    af = np.asarray(addr_from).astype(np.int64)
    at = np.asarray(addr_to).astype(np.int64)
    h_local = np.asarray(h_local, np.float32)
    x_local = np.asarray(x_local, np.float32)
    const = np.concatenate([np.asarray(h_global, np.float32).ravel(),
                            np.asarray(x_global, np.float32).ravel(),
                            np.asarray(t, np.float32).ravel()])  # [21]

    af2, at2, dr3, x3 = _prep(af, at, x_local)
    h_pad = np.zeros((NPAD, D), np.float32)
    h_pad[:N] = h_local

    if "nc" not in _BASS_CACHE:
        _BASS_CACHE["nc"] = _build_bass()
    nc = _BASS_CACHE["nc"]

    weights = {}
    for p, W0, b0, W1, b1, W2, b2 in (
        ("f", f_W0, f_b0, f_W1, f_b1, f_W2, f_b2),
        ("t", t_W0, t_b0, t_W1, t_b1, t_W2, t_b2),
    ):
        W0 = np.asarray(W0, np.float32)
        b0eff = np.asarray(b0, np.float32) + const @ W0[132:153]
        weights[p + "w0a"] = np.ascontiguousarray(W0[0:128])
        weights[p + "w0b"] = np.ascontiguousarray(W0[128:132])
        weights[p + "w1"] = np.asarray(W1, np.float32)
        weights[p + "w2"] = np.asarray(W2, np.float32)
        weights[p + "b0"] = b0eff.reshape(H, 1)
        weights[p + "b1"] = np.asarray(b1, np.float32).reshape(H, 1)
        weights[p + "b2"] = np.asarray(b2, np.float32).reshape(D, 1)

    in_maps = []
    for c in range(NCORES):
        m = {"hsh": np.ascontiguousarray(h_pad[c * NPC:(c + 1) * NPC]),
             "af2": af2[c], "at2": at2[c], "dr3": dr3[c], "x3": x3[c]}
        m.update(weights)
        in_maps.append(m)

    res = run_bass_kernel_spmd(nc, in_maps, core_ids=list(range(NCORES)),
                               trace=trace)
    out = np.concatenate(
        [res.results[c]["out3"].reshape(NPC, D) for c in range(NCORES)],
        axis=0)[:N]
    out = np.ascontiguousarray(out, dtype=np.float32)
    if trace:
        return out, res
    return out


def kernel(**inputs):
    try:
        return _kernel_bass(**inputs)
    except Exception:
        import traceback
        traceback.print_exc()
        return _kernel_numpy(**inputs)


# revision 9
# speedup vs baseline: 15.7122x; 1.3251x over previous
import numpy as np

# nn_LocalDynamics GNN message passing, full on-device pipeline.
#   delta[n] = sum_e tanh(fMLP(inp_e))[addr_from=n] + tanh(tMLP(inp_e))[addr_to=n]
#   out = tanh(delta);  inp_e = [h[from], h[to], x_e, hg, xg, t] (153 dims)
# Strategy: sort edges by destination node on host, shard by node range
# (12544 nodes/core).  Each core AllGathers h, gathers h-rows per edge via
# indirect DMA, runs the MLPs, and aggregates into its own output slice with
# eq-matrix matmuls (segmented sum) -- no scatter, no reduce collective.
N = 100_000
NPAD = 100_352
NPC = 12_544          # nodes per core
E = 800_000
D = 64
H = 128
WPC = 98              # windows (128 nodes) per core
CAP = 10              # chunks of 128 elems per (window, stream)
CH = 128
SW = CAP * CH         # 1280 elems per (window, stream)
TOTW = 2 * SW
TOT = WPC * TOTW      # 250880 padded stream elems per core
NCORES = 8


def _scatter_add(delta, idx, vals):
    o = np.argsort(idx, kind="stable")
    si = idx[o]
    sv = vals[o]
    starts = np.flatnonzero(np.r_[True, si[1:] != si[:-1]])
    sums = np.add.reduceat(sv, starts, axis=0)
    np.add.at(delta, si[starts], sums)


def _mlp_np(x, W0, b0, W1, b1, W2, b2):
    h = np.tanh(x @ W0 + b0)
    h = np.tanh(h @ W1 + b1)
    return h @ W2 + b2


def _kernel_numpy(addr_from, addr_to, h_local, h_global, x_local, x_global, t,
                  f_W0, f_b0, f_W1, f_b1, f_W2, f_b2,
                  t_W0, t_b0, t_W1, t_b1, t_W2, t_b2):
    af = np.asarray(addr_from).astype(np.int64)
    at = np.asarray(addr_to).astype(np.int64)
    h_local = np.asarray(h_local, dtype=np.float32)
    x_local = np.asarray(x_local, dtype=np.float32)
    const = np.concatenate([np.asarray(h_global, np.float32).ravel(),
                            np.asarray(x_global, np.float32).ravel(),
                            np.asarray(t, np.float32).ravel()])
    ne = af.shape[0]
    d_from = np.empty((ne, D), np.float32)
    d_to = np.empty((ne, D), np.float32)
    CHK = 100_000
    for s in range(0, ne, CHK):
        e = min(s + CHK, ne)
        inp = np.concatenate([h_local[af[s:e]], h_local[at[s:e]], x_local[s:e],
                              np.broadcast_to(const, (e - s, 21))], axis=1
                             ).astype(np.float32)
        d_from[s:e] = np.tanh(_mlp_np(inp, f_W0, f_b0, f_W1, f_b1, f_W2, f_b2))
        d_to[s:e] = np.tanh(_mlp_np(inp, t_W0, t_b0, t_W1, t_b1, t_W2, t_b2))
    delta = np.zeros((h_local.shape[0], D), dtype=np.float32)
    _scatter_add(delta, af, d_from)
    _scatter_add(delta, at, d_to)
    return np.tanh(delta).astype(np.float32)


def _prep(af, at, x):
    """Sort both streams by destination, pad into the fixed window layout.
    Returns per-core device arrays:
      af3/at3 [8, 98, 128, 20] int32, dr3 [8, 98, 128, 20] f32,
      x3 [8, 98, 4, 2560] f32.
    Chunk columns 0:10 = f-stream, 10:20 = t-stream; pad elems dr=-1, af/at=0.
    """
    AF = np.zeros((NCORES, TOT), np.int32)
    AT = np.zeros((NCORES, TOT), np.int32)
    DR = np.full((NCORES, TOT), -1.0, np.float32)
    X = np.zeros((NCORES, TOT, 4), np.float32)
    for si, dest in ((0, af), (1, at)):
        o = np.argsort(dest, kind="stable")
        ds_ = dest[o]
        w = ds_ >> 7
        cnts = np.bincount(w, minlength=NPAD // 128)
        if cnts.max() > SW:
            raise OverflowError(f"window count {cnts.max()} > {SW}")
        starts = np.concatenate([[0], np.cumsum(cnts)[:-1]])
        cc = np.arange(E, dtype=np.int64) - starts[w]
        core = w // WPC
        wl = w - core * WPC
        off = wl * TOTW + si * SW + cc
        AF[core, off] = af[o]
        AT[core, off] = at[o]
        DR[core, off] = (ds_ & 127).astype(np.float32)
        X[core, off] = x[o]
    af2 = np.ascontiguousarray(
        AF.reshape(NCORES, TOT // CH, CH).transpose(0, 2, 1))
    at2 = np.ascontiguousarray(
        AT.reshape(NCORES, TOT // CH, CH).transpose(0, 2, 1))
    import ml_dtypes
    dr3 = np.ascontiguousarray(
        DR.reshape(NCORES, WPC, 2 * CAP, CH).transpose(0, 1, 3, 2)
    ).astype(ml_dtypes.bfloat16)
    x3 = np.ascontiguousarray(
        X.reshape(NCORES, WPC, TOTW, 4).transpose(0, 1, 3, 2)
    ).astype(ml_dtypes.bfloat16)
    return af2, at2, dr3, x3


_BASS_CACHE = {}


def _patch_walrus(bass, mybir, tile):
    # walrus in this env rejects Drain instructions carrying >1 sem wait;
    # move each wait onto its own sync nop before the drain.
    def _patched(self, tick_clock, wait_clock):
        from concourse.tile import ScopedClock
        nop0 = self.nc.sync.nop(nofuse=True)
        wait_clock.add_sem_waits(nop0.ins, ScopedClock({None: tick_clock.global_clock}))
        si = nop0.ins.sync_info
        if si is not None and si.on_wait and len(si.on_wait) > 1:
            waits = list(si.on_wait)
            si.on_wait = waits[:1]
            for w in waits[1:]:
                n = self.nc.sync.nop(nofuse=True)
                n.ins.sync_info = mybir.SyncInfo(on_wait=[w], on_update=[])
        self.nc.sync.drain()
        self.nc.all_engine_barrier()
        popped = self.nc._tile_sem_poison_stack.pop()
        assert popped is self._sem_poison
        self.nc.clear_and_free_semaphores(list(self.sems.allocated().values()))
        self.nc.all_engine_barrier()

    tile.TileContext._drain_and_barrier = _patched


def _split_multi_waits(nc, mybir):
    # this walrus rejects any compute instruction carrying >1 sem wait;
    # hoist extra waits onto same-engine nops placed just before it.
    ctr = 0
    for bb in nc.main_func.blocks:
        new = []
        for ins in bb.instructions:
            si = getattr(ins, "sync_info", None)
            if si is not None and si.on_wait and len(si.on_wait) > 1:
                waits = list(si.on_wait)
                si.on_wait = [waits[-1]]
                for w in waits[:-1]:
                    ctr += 1
                    nop = mybir.InstNoOp(
                        name=f"wsplit-{ctr}", engine=ins.engine, ins=[], outs=[],
                        sync_info=mybir.SyncInfo(on_wait=[w], on_update=[]))
                    new.append(nop)
            new.append(ins)
        bb.instructions[:] = new


def _build_bass():
    import concourse.bass as bass
    import concourse.mybir as mybir
    import concourse.tile as tile
    from concourse.bass import ds, IndirectOffsetOnAxis

    _patch_walrus(bass, mybir, tile)

    f32 = mybir.dt.float32
    bf16 = mybir.dt.bfloat16
    i32 = mybir.dt.int32
    nc = bass.Bass(num_devices=NCORES)

    h_in = nc.dram_tensor("hsh", [NPC, D], f32, kind="ExternalInput")
    af2 = nc.dram_tensor("af2", [CH, WPC * 2 * CAP], i32, kind="ExternalInput")
    at2 = nc.dram_tensor("at2", [CH, WPC * 2 * CAP], i32, kind="ExternalInput")
    dr3 = nc.dram_tensor("dr3", [WPC, CH, 2 * CAP], bf16, kind="ExternalInput")
    x3 = nc.dram_tensor("x3", [WPC, 4, TOTW], bf16, kind="ExternalInput")
    wts = {}
    for p in ("f", "t"):
        wts[p + "w0a"] = nc.dram_tensor(p + "w0a", [128, H], f32, kind="ExternalInput")
        wts[p + "w0b"] = nc.dram_tensor(p + "w0b", [4, H], bf16, kind="ExternalInput")
        wts[p + "w1"] = nc.dram_tensor(p + "w1", [H, H], f32, kind="ExternalInput")
        wts[p + "w2"] = nc.dram_tensor(p + "w2", [H, D], f32, kind="ExternalInput")
        wts[p + "b0"] = nc.dram_tensor(p + "b0", [H, 1], f32, kind="ExternalInput")
        wts[p + "b1"] = nc.dram_tensor(p + "b1", [H, 1], f32, kind="ExternalInput")
        wts[p + "b2"] = nc.dram_tensor(p + "b2", [D, 1], f32, kind="ExternalInput")
    outp = nc.dram_tensor("out3", [WPC, CH, D], bf16, kind="ExternalOutput")

    h_int = nc.dram_tensor("h_int", [NPC, D], f32, kind="Internal")
    h_full = nc.dram_tensor("h_full", [NPAD, D], f32, kind="Internal",
                            addr_space="Shared")
    # DRAM staging for gathered h-rows: SWDGE indirect DMAs must stay outside
    # For_i (this walrus can't encode the loop's SWDGE sem-reset), so an
    # unrolled gather phase writes here and the loop reads it back with HWDGE.
    gstage = nc.dram_tensor("gstage", [WPC, CH, 2 * CAP, CH], f32,
                            kind="Internal")
    ident_dr = nc.inline_tensor(np.eye(CH, dtype=np.float32), name="ident")
    import ml_dtypes
    iota_dr = nc.inline_tensor(
        np.ascontiguousarray(np.broadcast_to(
            np.arange(CH), (CH, CH))).astype(ml_dtypes.bfloat16), name="iotam")

    Tanh = mybir.ActivationFunctionType.Tanh
    with tile.TileContext(nc) as tc:
        with tc.tile_pool(name="wp", bufs=1) as wp, \
             tc.tile_pool(name="io", bufs=2) as io, \
             tc.tile_pool(name="act", bufs=2) as ap_, \
             tc.tile_pool(name="psm", bufs=2, space="PSUM") as psm, \
             tc.tile_pool(name="psw", bufs=1, space="PSUM") as psw:
            wt = {}
            for k, drh in wts.items():
                tl = wp.tile(list(drh.shape), drh.dtype, tag="w" + k)
                nc.sync.dma_start(out=tl[:], in_=drh[:])
                wt[k] = tl
            ident = wp.tile([CH, CH], f32, tag="ident")
            nc.sync.dma_start(out=ident[:], in_=ident_dr[:])
            iota_sb = wp.tile([CH, CH], bf16, tag="iota")
            nc.sync.dma_start(out=iota_sb[:], in_=iota_dr[:])

            nc.sync.dma_start(out=h_int[:], in_=h_in[:])
            nc.gpsimd.collective_compute(
                "AllGather", mybir.AluOpType.bypass,
                replica_groups=[list(range(NCORES))],
                ins=[h_int[:]], outs=[h_full[:]])

            af_all = wp.tile([CH, WPC * 2 * CAP], i32, tag="afall")
            at_all = wp.tile([CH, WPC * 2 * CAP], i32, tag="atall")
            nc.sync.dma_start(out=af_all[:], in_=af2[:])
            nc.sync.dma_start(out=at_all[:], in_=at2[:])

            for w in range(WPC):
                g2 = io.tile([CH, 2 * CAP, CH], f32, tag="g2")
                for c in range(2 * CAP):
                    col = w * 2 * CAP + c
                    # the vector-indirect DMA only supports one index per
                    # partition per instruction (idx [128,1] -> out [128,64])
                    nc.gpsimd.indirect_dma_start(
                        out=g2[:, c, 0:64], out_offset=None, in_=h_full[:],
                        in_offset=IndirectOffsetOnAxis(
                            ap=af_all[:, col:col + 1], axis=0))
                    nc.gpsimd.indirect_dma_start(
                        out=g2[:, c, 64:128], out_offset=None, in_=h_full[:],
                        in_offset=IndirectOffsetOnAxis(
                            ap=at_all[:, col:col + 1], axis=0))
                nc.sync.dma_start(out=gstage[w], in_=g2[:])

            with tc.For_i(0, WPC, 1) as iv:
                dr_sb = io.tile([CH, 2 * CAP], bf16, tag="dr")
                x_sb = io.tile([4, TOTW], bf16, tag="x")
                gld = io.tile([CH, 2 * CAP, CH], f32, tag="gld")
                nc.sync.dma_start(out=dr_sb[:], in_=dr3[ds(iv, 1)].squeeze(0))
                nc.sync.dma_start(out=x_sb[:], in_=x3[ds(iv, 1)].squeeze(0))
                nc.sync.dma_start(out=gld[:], in_=gstage[ds(iv, 1)].squeeze(0))
                win_ps = psw.tile([CH, D], f32, tag="win")
                for si, p in enumerate(("f", "t")):
                    inpT = psm.tile([CH, SW], f32, tag="mlp")
                    for k in range(CAP):
                        nc.tensor.transpose(
                            out=inpT[:, k * CH:(k + 1) * CH],
                            in_=gld[:, si * CAP + k, :], identity=ident[:])
                    inp_sb = ap_.tile([CH, SW], f32, tag="inp")
                    nc.vector.tensor_copy(out=inp_sb[:], in_=inpT[:])
                    ps0 = psm.tile([CH, SW], f32, tag="mlp")
                    for lo in range(0, SW, 512):
                        hi = min(lo + 512, SW)
                        nc.tensor.matmul(out=ps0[:, lo:hi], lhsT=wt[p + "w0a"][:],
                                         rhs=inp_sb[:, lo:hi], start=True, stop=False)
                        nc.tensor.matmul(out=ps0[:, lo:hi], lhsT=wt[p + "w0b"][:],
                                         rhs=x_sb[:, si * SW + lo:si * SW + hi],
                                         start=False, stop=True)
                    h1 = ap_.tile([CH, SW], f32, tag="h1")
                    nc.scalar.activation(h1[:], ps0[:], Tanh, bias=wt[p + "b0"][:, 0:1])
                    ps1 = psm.tile([CH, SW], f32, tag="mlp")
                    for lo in range(0, SW, 512):
                        hi = min(lo + 512, SW)
                        nc.tensor.matmul(out=ps1[:, lo:hi], lhsT=wt[p + "w1"][:],
                                         rhs=h1[:, lo:hi], start=True, stop=True)
                    h2 = ap_.tile([CH, SW], f32, tag="h2")
                    nc.scalar.activation(h2[:], ps1[:], Tanh, bias=wt[p + "b1"][:, 0:1])
                    psd = psm.tile([CH, SW], f32, tag="mlp")
                    for lo in range(0, SW, 512):
                        hi = min(lo + 512, SW)
                        nc.tensor.matmul(out=psd[0:D, lo:hi], lhsT=wt[p + "w2"][:],
                                         rhs=h2[:, lo:hi], start=True, stop=True)
                    d_fm = ap_.tile([D, SW], f32, tag="dfm")
                    nc.scalar.activation(d_fm[:], psd[0:D, :], Tanh,
                                         bias=wt[p + "b2"][:, 0:1])
                    dT = psm.tile([CH, SW], f32, tag="mlp")
                    for k in range(CAP):
                        nc.tensor.transpose(
                            out=dT[:, k * D:(k + 1) * D],
                            in_=d_fm[:, k * CH:(k + 1) * CH],
                            identity=ident[0:64, 0:64])
                    d_em = ap_.tile([CH, CAP * D], f32, tag="dem")
                    nc.vector.tensor_copy(out=d_em[:], in_=dT[:, 0:CAP * D])
                    eq_sb = ap_.tile([CH, SW], f32, tag="eq")
                    for k in range(CAP):
                        col = si * CAP + k
                        nc.vector.tensor_tensor(
                            out=eq_sb[:, k * CH:(k + 1) * CH],
                            in0=dr_sb[:, col:col + 1].to_broadcast([CH, CH]),
                            in1=iota_sb[:], op=mybir.AluOpType.is_equal)
                    for k in range(CAP):
                        nc.tensor.matmul(
                            out=win_ps[:], lhsT=eq_sb[:, k * CH:(k + 1) * CH],
                            rhs=d_em[:, k * D:(k + 1) * D],
                            start=(si == 0 and k == 0),
                            stop=(si == 1 and k == CAP - 1))
                out_sb = ap_.tile([CH, D], bf16, tag="out")
                nc.scalar.activation(out_sb[:], win_ps[:], Tanh)
                nc.sync.dma_start(out=outp[ds(iv, 1)].squeeze(0), in_=out_sb[:])

    _split_multi_waits(nc, mybir)
    return nc


def _kernel_bass(addr_from, addr_to, h_local, h_global, x_local, x_global, t,
                 f_W0, f_b0, f_W1, f_b1, f_W2, f_b2,
                 t_W0, t_b0, t_W1, t_b1, t_W2, t_b2, trace=False):
    import sys
    if "/opt/trn_rl_repo" not in sys.path:
        sys.path.insert(0, "/opt/trn_rl_repo")
    from concourse.bass_utils import run_bass_kernel_spmd

    af = np.asarray(addr_from).astype(np.int64)
    at = np.asarray(addr_to).astype(np.int64)
    h_local = np.asarray(h_local, np.float32)
    x_local = np.asarray(x_local, np.float32)
    const = np.concatenate([np.asarray(h_global, np.float32).ravel(),
                            np.asarray(x_global, np.float32).ravel(),
                            np.asarray(t, np.float32).ravel()])  # [21]

    af2, at2, dr3, x3 = _prep(af, at, x_local)
    h_pad = np.zeros((NPAD, D), np.float32)
    h_pad[:N] = h_local

    if "nc" not in _BASS_CACHE:
        _BASS_CACHE["nc"] = _build_bass()
    nc = _BASS_CACHE["nc"]

    weights = {}
    for p, W0, b0, W1, b1, W2, b2 in (
        ("f", f_W0, f_b0, f_W1, f_b1, f_W2, f_b2),
        ("t", t_W0, t_b0, t_W1, t_b1, t_W2, t_b2),
    ):
        W0 = np.asarray(W0, np.float32)
        b0eff = np.asarray(b0, np.float32) + const @ W0[132:153]
        weights[p + "w0a"] = np.ascontiguousarray(W0[0:128])
        import ml_dtypes
        weights[p + "w0b"] = np.ascontiguousarray(W0[128:132]).astype(
            ml_dtypes.bfloat16)
        weights[p + "w1"] = np.asarray(W1, np.float32)
        weights[p + "w2"] = np.asarray(W2, np.float32)
        weights[p + "b0"] = b0eff.reshape(H, 1)
        weights[p + "b1"] = np.asarray(b1, np.float32).reshape(H, 1)
        weights[p + "b2"] = np.asarray(b2, np.float32).reshape(D, 1)

    in_maps = []
    for c in range(NCORES):
        m = {"hsh": np.ascontiguousarray(h_pad[c * NPC:(c + 1) * NPC]),
             "af2": af2[c], "at2": at2[c], "dr3": dr3[c], "x3": x3[c]}
        m.update(weights)
        in_maps.append(m)

    res = run_bass_kernel_spmd(nc, in_maps, core_ids=list(range(NCORES)),
                               trace=trace)
    out = np.concatenate(
        [np.asarray(res.results[c]["out3"], np.float32).reshape(NPC, D)
         for c in range(NCORES)], axis=0)[:N]
    out = np.ascontiguousarray(out, dtype=np.float32)
    if trace:
        return out, res
    return out


def kernel(**inputs):
    try:
        return _kernel_bass(**inputs)
    except Exception:
        import traceback
        traceback.print_exc()
        return _kernel_numpy(**inputs)
